# revision 65
# baseline (speedup 1.0000x reference)
"""AdaptiveGraphStructure Bass kernel for 8 TRN2 NeuronCores.

Math (per batch b):
  u[i,h] = emb[i] @ Wi.T + b1        (Wi = W1[:, :128])
  v[j,h] = emb[j] @ Wj.T             (Wj = W1[:, 128:])
  e[i,j] = W2 . relu(u[i] + v[j])    (+b2, dropped: softmax-invariant)
  masked with visited[i] | visited[j], then row softmax.

Structure exploited (exactly matching reference fp32 semantics):
  - visited rows come out uniform 1/N (softmax of a constant row).
  - visited columns in unvisited rows come out exactly 0.0 (exp
    underflow of -1e9 - max in fp32).
So the device only computes the [unvisited x unvisited] block per
batch; the host scatters it into the full output, zero-fills masked
columns and writes 1/N rows for visited i. This roughly quarters the
dominant B*N*N*H work (p(visited)=0.5).

Device scheme per core (PE matmul outputs must be 32-aligned):
  RC rows in groups of 32; h=64 split into 16 chunks of 4.
  VC[c] [128=(i_sub 32 x k 4), JPAD]: v[j, 4c+k] replicated over i_sub,
     built by matmul with host-replicated Wj columns (wjrep, bf16).
  For group g, chunk c:
     R = relu(VC[c] + UC[:, g*16+c])  on DVE (tensor_scalar add+max, bf16 4x)
     psum_e[g*32:+32, :] += w2stack[c].T @ R   (K=128, M=32, bf16)
  pad-column mask (-1e9) added first via ones(1x32) x madd K=1 matmul.
  softmax: DVE row-max, ACT exp with accumulated row-sum,
  DVE reciprocal, ACT copy*scale, DMA out.

Sharding: per batch, the unvisited rows are split over 4 cores
(cores 0-3: batch 0, cores 4-7: batch 1), padded to RC rows each.
All 8 cores run one SPMD program; no collectives.
"""

from contextlib import ExitStack

import ml_dtypes
import numpy as np

import concourse.tile as tile
from concourse import bacc, mybir
from concourse.bass_utils import run_bass_kernel_spmd

B, N, D = 2, 1024, 128
H = D // 2  # 64
NCH = H // 4  # 16 h-chunks

F32 = mybir.dt.float32
BF16 = mybir.dt.bfloat16

_CACHE = {}


def _build_nc(RC, JPAD, reps=1):
    """RC: padded rows per core (multiple of 32). JPAD: padded j (mult 128).

    reps>1 repeats the whole compute (benchmark builds only)."""
    NG = RC // 32  # row groups per core
    jchunks = []
    o = 0
    while o < JPAD:
        ln = min(512, JPAD - o)
        jchunks.append((o, ln))
        o += ln

    NPE = 4  # VC chunks built via PE matmul (pipeline head); rest via DMA

    nc = bacc.Bacc("TRN2", target_bir_lowering=False, num_devices=8)
    UC = nc.dram_tensor("UC", [128, NG * NCH], F32, kind="ExternalInput")
    embT_jc = nc.dram_tensor("embT_jc", [D, JPAD], BF16, kind="ExternalInput")
    wjrep = nc.dram_tensor("wjrep", [D, NPE, 128], BF16, kind="ExternalInput")
    wjT = nc.dram_tensor("wjT", [D, H], BF16, kind="ExternalInput")
    w2stack = nc.dram_tensor(
        "w2stack", [128, NCH * 32], F32, kind="ExternalInput"
    )
    out = nc.dram_tensor("out", [RC, JPAD], F32, kind="ExternalOutput")

    with tile.TileContext(nc) as tc, ExitStack() as ctx:
        const = ctx.enter_context(tc.tile_pool(name="const", bufs=1))
        rpool = ctx.enter_context(tc.tile_pool(name="r", bufs=12))
        epool = ctx.enter_context(tc.tile_pool(name="e", bufs=2))
        spool = ctx.enter_context(tc.tile_pool(name="stats", bufs=4))
        psum_e_pool = ctx.enter_context(
            tc.tile_pool(
                name="psum_e",
                bufs=2 if RC > 128 else 1,
                space="PSUM",
            )
        )
        psum_v_pool = ctx.enter_context(
            tc.tile_pool(
                name="psum_v",
                bufs=4 if RC > 128 else 6,
                space="PSUM",
            )
        )

        # ---- load constants (spread across DMA issue paths) ----
        embT_jc_sb = const.tile([D, JPAD], BF16)
        wjrep_sb = const.tile([D, NPE, 128], BF16)
        vh0 = JPAD // 2
        nc.sync.dma_start(embT_jc_sb[:, 0:vh0], embT_jc[:, 0:vh0])
        nc.sync.dma_start(wjrep_sb[:, 0, :], wjrep[:, 0, :])
        nc.sync.dma_start(embT_jc_sb[:, vh0:], embT_jc[:, vh0:])
        nc.sync.dma_start(wjrep_sb[:, 1:, :], wjrep[:, 1:, :])
        wjT_sb = const.tile([D, H], BF16)
        nc.sync.dma_start(wjT_sb[:], wjT[:])
        UC_sb = const.tile([128, NG * NCH], F32)
        nc.scalar.dma_start(UC_sb[:], UC[:])
        w2s_f32 = const.tile([128, NCH * 32], F32)
        nc.scalar.dma_start(w2s_f32[:], w2stack[:])
        w2stack_sb = const.tile([128, NCH * 32], BF16)
        nc.vector.tensor_copy(w2stack_sb[:], w2s_f32[:])


        # ---- VC[c] [128=(i_sub,k), JPAD] = v[j, 4c+k] replicated.
        # Built lazily (just-in-time inside the first row tile) so DVE/PE
        # pipeline from the start instead of waiting for all 16 tiles.
        VC = [None] * NCH

        vh = JPAD // 2
        v_sb = const.tile([H, JPAD], BF16)

        def build_vsb():
            # v_sb [64h, JPAD] = Wj @ embT, the master copy VC tiles
            # replicate from.
            for o in (0, vh):
                psum_vh = psum_v_pool.tile([H, vh], F32, tag="psum_vc")
                nc.tensor.matmul(
                    psum_vh[:],
                    wjT_sb[:],
                    embT_jc_sb[:, o : o + vh],
                    start=True,
                    stop=True,
                )
                nc.scalar.copy(v_sb[:, o : o + vh], psum_vh[:])

        def build_vc(c):
            vc = const.tile([128, JPAD], BF16, tag=f"vc{c}")
            if c < NPE:
                # PE path (pipeline head): column halves, 1 PSUM bank each
                for o in (0, vh):
                    psum_vc = psum_v_pool.tile([128, vh], F32, tag="psum_vc")
                    nc.tensor.matmul(
                        psum_vc[:],
                        wjrep_sb[:, c, :],
                        embT_jc_sb[:, o : o + vh],
                        start=True,
                        stop=True,
                    )
                    nc.scalar.copy(vc[:, o : o + vh], psum_vc[:])
            else:
                # replicate rows 4c..4c+3 of v_sb across 32 partitions each
                src = (
                    v_sb[4 * c : 4 * c + 4, :]
                    .unsqueeze(1)
                    .broadcast_to([4, 32, JPAD])
                )
                nc.sync.dma_start(vc[:], src)
            VC[c] = vc

        # ---- main loop over row tiles of <=128 ----
        row_tiles = []
        r = 0
        while r < RC:
            h_ = min(128, RC - r)
            row_tiles.append((r, h_))
            r += h_
        all_tiles = row_tiles * reps
        for it, (r0, th) in enumerate(all_tiles):
            psum_e = psum_e_pool.tile([128, JPAD], F32, tag="psum_e")
            ngr = th // 32
            # c outer, groups inner: each VC chunk's build+drain amortizes
            # over ngr consuming matmuls and stays hidden behind PE.
            if VC[0] is None:
                build_vc(0)
                build_vc(1)
                build_vsb()
                build_vc(2)
                build_vc(3)
            PRE = 3
            for c in range(NCH):
                for cc in (c, c + PRE):
                    if cc < NCH and VC[cc] is None:
                        build_vc(cc)
                for g4 in range(ngr):
                    g = (r0 // 32) + g4
                    rows = slice(g4 * 32, (g4 + 1) * 32)
                    R = rpool.tile([128, JPAD], BF16)
                    # spread R production: gpsimd takes one group per chunk
                    eng = nc.gpsimd if (ngr == 4 and g4 == 3) else nc.vector
                    eng.tensor_scalar(
                        R[:],
                        VC[c][:],
                        UC_sb[:, g * NCH + c : g * NCH + c + 1],
                        0.0,
                        mybir.AluOpType.add,
                        mybir.AluOpType.max,
                    )
                    for (o, ln) in jchunks:
                        nc.tensor.matmul(
                            psum_e[rows, o : o + ln],
                            w2stack_sb[:, c * 32 : (c + 1) * 32],
                            R[:, o : o + ln],
                            start=(c == 0),
                            stop=(c == NCH - 1),
                            skip_group_check=True,
                            tile_position=(0, g4 * 32),
                        )

            # ---- softmax over free dim ----
            last = it == len(all_tiles) - 1
            pe = psum_e[0:th, :]
            mx = spool.tile([128, 1], F32, tag="mx")
            nc.vector.tensor_reduce(
                mx[0:th, :], pe, mybir.AxisListType.X, mybir.AluOpType.max
            )
            nbias = spool.tile([128, 1], F32, tag="nbias")
            nc.vector.tensor_scalar(
                nbias[0:th, :],
                mx[0:th, :],
                -1.0,
                None,
                mybir.AluOpType.mult,
            )
            E = epool.tile([128, JPAD], F32, tag="E")
            sm = spool.tile([128, 1], F32, tag="sm")
            nc.scalar.activation(
                E[0:th, :],
                pe,
                mybir.ActivationFunctionType.Exp,
                bias=nbias[0:th, 0:1],
                accum_out=sm[0:th, :],
            )
            out_sb = epool.tile([128, JPAD], F32, tag="out_sb")
            if not last:
                rs = spool.tile([128, 1], F32, tag="rs")
                nc.vector.reciprocal(rs[0:th, :], sm[0:th, :])
                # normalize + store split by row halves: DVE and ACT in
                # parallel; each half's DMA overlaps the other's compute.
                h0 = (th // 2 + 31) // 32 * 32 if th > 32 else th
                h0 = min(h0, th)
                if h0 < th:
                    nc.scalar.activation(
                        out_sb[h0:th, :],
                        E[h0:th, :],
                        mybir.ActivationFunctionType.Copy,
                        scale=rs[h0:th, 0:1],
                    )
                    nc.sync.dma_start(
                        out[r0 + h0 : r0 + th, :], out_sb[h0:th, :]
                    )
                nc.vector.tensor_scalar(
                    out_sb[0:h0, :],
                    E[0:h0, :],
                    rs[0:h0, 0:1],
                    None,
                    mybir.AluOpType.mult,
                )
                nc.sync.dma_start(out[r0 : r0 + h0, :], out_sb[0:h0, :])
            else:
                # last tile: normalize column halves on DVE and ACT in
                # parallel; DMAs go to separate queues so they overlap too.
                rs = spool.tile([128, 1], F32, tag="rs")
                nc.vector.reciprocal(rs[0:th, :], sm[0:th, :])
                sl = (JPAD // 2 + 31) // 32 * 32
                nc.vector.tensor_scalar(
                    out_sb[0:th, 0:sl],
                    E[0:th, 0:sl],
                    rs[0:th, 0:1],
                    None,
                    mybir.AluOpType.mult,
                )
                nc.sync.dma_start(
                    out[r0 : r0 + th, 0:sl], out_sb[0:th, 0:sl]
                )
                if sl < JPAD:
                    nc.scalar.activation(
                        out_sb[0:th, sl:],
                        E[0:th, sl:],
                        mybir.ActivationFunctionType.Copy,
                        scale=rs[0:th, 0:1],
                    )
                    nc.scalar.dma_start(
                        out[r0 : r0 + th, sl:], out_sb[0:th, sl:]
                    )

    nc.compile()
    return nc


def _get_nc(RC, JPAD):
    key = (RC, JPAD)
    if key not in _CACHE:
        _CACHE[key] = _build_nc(RC, JPAD)
    return _CACHE[key]


def kernel(
    node_embeddings,
    visited,
    remaining_capacity,
    W1,
    b1,
    W2,
    b2,
    _trace=False,
):
    node_embeddings = np.asarray(node_embeddings, dtype=np.float32)
    visited = np.asarray(visited).astype(bool)
    W1 = np.asarray(W1, dtype=np.float32)
    b1 = np.asarray(b1, dtype=np.float32)
    W2 = np.asarray(W2, dtype=np.float32)

    WiT = np.ascontiguousarray(W1[:, :D].T)  # [D, H]
    WjT = np.ascontiguousarray(W1[:, D:].T)  # [D, H]

    unvis = [np.flatnonzero(~visited[b]) for b in range(B)]
    jc = [len(u) for u in unvis]
    jcmax = max(max(jc), 1)
    # Cap device rows at 512/batch (128/core) when the overflow is small:
    # group costs are free-dim-bound, so a 130th row per core would cost a
    # whole extra 32-row group. The few overflow rows are computed on host.
    cap = [jc[b] if not (512 < jc[b] <= 576) else 512 for b in range(B)]
    q = [max((cap[b] + 3) // 4, 1) for b in range(B)]  # rows per core
    RC = max(32, ((max(q) + 31) // 32) * 32)
    JPAD = max(128, ((jcmax + 31) // 32) * 32)
    NG = RC // 32

    # k-major partition packing: p = k*32 + i_sub (so VC replication from
    # v_sb is a contiguous-partition broadcast DMA).
    # wjrep[d, c, (k, i_sub)] = WjT[d, 4c+k]   (PE-path chunks only)
    NPE = 4
    wjrep = np.ascontiguousarray(
        np.broadcast_to(
            WjT.reshape(D, NCH, 4, 1), (D, NCH, 4, 32)
        ).reshape(D, NCH, 128)[:, :NPE]
    ).astype(ml_dtypes.bfloat16)

    # w2stack[(k, i_sub), c*32 + i'] = W2[4c+k] * (i_sub == i')
    W2r = W2[0].reshape(NCH, 4)  # [c, k]
    ws = np.zeros((4, 32, NCH, 32), dtype=np.float32)
    for i_ in range(32):
        ws[:, i_, :, i_] = W2r.T  # [k, c]
    w2stack = ws.reshape(128, NCH * 32)

    # Pad j-columns get an embedding whose projection v_pad = Wj @ emb_pad
    # is -V0 on positive-W2 channels and +V0 on negative ones, making the
    # pad logit ~ -V0 * sum|W2^-| << -100; exp underflows to exactly 0 so
    # pads drop out of the softmax (same effect as an explicit -1e9 mask).
    w2v = W2[0].astype(np.float64)
    neg_mass = float(np.abs(w2v[w2v < 0]).sum())
    V0 = float(np.clip(400.0 / max(neg_mass, 1e-3), 256.0, 1e6))
    vp_target = np.where(w2v >= 0, -V0, V0)
    Wj64 = W1[:, D:].astype(np.float64)
    emb_pad = (Wj64.T @ np.linalg.solve(Wj64 @ Wj64.T, vp_target)).astype(
        np.float32
    )

    in_maps = []
    for cid in range(8):
        b = cid // 4
        k = cid % 4
        rows = unvis[b][: cap[b]][k * q[b] : (k + 1) * q[b]]
        nr = len(rows)
        emb_i = np.zeros((RC, D), dtype=np.float32)
        if nr:
            emb_i[:nr] = node_embeddings[b, rows]
        u = emb_i @ WiT + b1  # [RC, H]
        UC = np.ascontiguousarray(
            u.reshape(NG, 32, NCH, 4)
            .transpose(3, 1, 0, 2)
            .reshape(128, NG * NCH)
        ).astype(np.float32)
        embT_jc = np.zeros((D, JPAD), dtype=ml_dtypes.bfloat16)
        embT_jc[:, : jc[b]] = node_embeddings[b, unvis[b]].T
        if jc[b] < JPAD:
            embT_jc[:, jc[b] :] = emb_pad[:, None]
        in_maps.append(
            {
                "UC": UC,
                "embT_jc": embT_jc,
                "wjrep": wjrep,
                "wjT": WjT.astype(ml_dtypes.bfloat16),
                "w2stack": w2stack,
            }
        )

    nc = _get_nc(RC, JPAD)
    _CACHE["last_in_maps"] = in_maps
    _CACHE["last_nc"] = nc
    res = run_bass_kernel_spmd(
        nc, in_maps, core_ids=list(range(8)), trace=_trace
    )
    _CACHE["last_result"] = res

    out = np.zeros((B, N, N), dtype=np.float32)
    for b in range(B):
        out[b, visited[b], :] = np.float32(1.0 / N)
    for cid in range(8):
        b = cid // 4
        k = cid % 4
        rows = unvis[b][: cap[b]][k * q[b] : (k + 1) * q[b]]
        nr = len(rows)
        if nr == 0:
            continue
        blk = res.results[cid]["out"][:nr, : jc[b]]
        out[b, rows[:, None], unvis[b][None, :]] = blk
    # overflow rows (device capacity cap) computed on host, exactly
    for b in range(B):
        rows = unvis[b][cap[b] :]
        if len(rows) == 0:
            continue
        v = node_embeddings[b, unvis[b]] @ WjT  # [jc, H]
        u = node_embeddings[b, rows] @ WiT + b1  # [nh, H]
        e = np.maximum(u[:, None, :] + v[None, :, :], 0.0) @ W2[0]
        e -= e.max(axis=1, keepdims=True)
        p = np.exp(e)
        p /= p.sum(axis=1, keepdims=True)
        out[b, rows[:, None], unvis[b][None, :]] = p.astype(np.float32)
    return out
